# revision 5
# baseline (speedup 1.0000x reference)
"""ExpEig kernel for Trainium2: X = U diag(exp(w)) U^T = expm(P).

Since P is SPD and U diag(exp(w)) U^T is exactly the matrix exponential of P,
we compute expm(P) per 32x32 matrix with scaling-and-squaring:
    Y = P / 2^s  (fp16)
    T = sum_{j=0}^{m} Y^j / j!   (Horner, fp16 operands, fp32 PSUM accumulate)
    X = T^(2^s)  (s squarings, last one emitted in fp32)

Data-parallel over 8 NeuronCores (256 batch rows each -> 4096 matrices/core).
On each core the 4096 matrices are split across 4 partition groups of 32
partitions; matmuls run on the 4 diagonal 32x32 PE tiles (tile_position is
auto-derived from base partitions), PSUM is evacuated with 128-partition-wide
vector/scalar instructions.
"""

import math
import os
import sys

sys.path.insert(0, "/opt/trn_rl_repo")

import numpy as np

import concourse.bacc as bacc
import concourse.bass as bass
import concourse.mybir as mybir
import concourse.tile as tile
from concourse.bass_utils import run_bass_kernel_spmd

# Problem shape (hardcoded per spec)
B, H, NN = 2048, 16, 32
NCORES = 8
BPC = B // NCORES            # 256 batch rows per core
NMAT = BPC * H               # 4096 matrices per core
NGRP = NMAT // 4             # 1024 matrices per partition group

# Algorithm parameters
M_TAYLOR = int(os.environ.get("EXPEIG_M", "7"))   # Taylor degree
S_SQ = int(os.environ.get("EXPEIG_S", "3"))       # squarings

# Tiling
CHUNK_G = 64                 # matrices per group per chunk
CW = CHUNK_G * NN            # free-dim columns per chunk (2048)
SUB = 16                     # matrices per evacuation batch per group
NSUB = CHUNK_G // SUB        # 4
SW = SUB * NN                # 512 cols per evac batch

F16 = mybir.dt.float16
F32 = mybir.dt.float32


def build_nc(nchunks):
    nc = bacc.Bacc("TRN2", target_bir_lowering=False)
    x_dram = nc.dram_tensor("x", [128, nchunks * CW], F32, kind="ExternalInput")
    c_dram = nc.dram_tensor("consts", [128, (M_TAYLOR + 1) * NN], F16,
                            kind="ExternalInput")
    y_dram = nc.dram_tensor("y", [128, nchunks * CW], F32, kind="ExternalOutput")

    with tile.TileContext(nc) as tc:
        with (
            tc.tile_pool(name="const", bufs=1) as constp,
            tc.tile_pool(name="io", bufs=3) as iop,
            tc.tile_pool(name="work", bufs=5) as workp,
            tc.tile_pool(name="psum", bufs=6, space=bass.MemorySpace.PSUM) as psump,
        ):
            consts = constp.tile([128, (M_TAYLOR + 1) * NN], F16)
            nc.sync.dma_start(consts[:], c_dram[:])

            for ch in range(nchunks):
                xin = iop.tile([128, CW], F32, tag="xin")
                nc.sync.dma_start(xin[:], x_dram[:, ch * CW:(ch + 1) * CW])

                # Y = fp16(X * 2^-s) on ScalarE
                y16 = workp.tile([128, CW], F16, tag="y16")
                for i in range(CW // 512):
                    nc.scalar.activation(
                        y16[:, i * 512:(i + 1) * 512],
                        xin[:, i * 512:(i + 1) * 512],
                        mybir.ActivationFunctionType.Copy,
                        scale=float(0.5 ** S_SQ),
                    )

                # Horner: R <- Y @ R + a_j I, j = m-1 .. 0; R init = a_m I
                Rcur = None
                for j in range(M_TAYLOR - 1, -1, -1):
                    Rnew = workp.tile([128, CW], F16, tag="R")
                    for sb in range(NSUB):
                        ps = psump.tile([128, SW], F32, tag="ps")
                        for i in range(SUB):
                            col = (sb * SUB + i) * NN
                            for g in range(4):
                                gp = slice(g * 32, (g + 1) * 32)
                                if Rcur is None:
                                    rhs = consts[gp, M_TAYLOR * NN:(M_TAYLOR + 1) * NN]
                                else:
                                    rhs = Rcur[gp, col:col + NN]
                                nc.tensor.matmul(
                                    ps[gp, i * NN:(i + 1) * NN],
                                    y16[gp, col:col + NN],
                                    rhs,
                                    start=True, stop=True,
                                    tile_position=(g * 32, g * 32),
                                )
                        # evac: Rnew = ps + a_j I  (const tile broadcast over SUB)
                        cj = consts[:, j * NN:(j + 1) * NN]
                        nc.vector.tensor_tensor(
                            Rnew[:, sb * SW:(sb + 1) * SW].rearrange(
                                "p (i c) -> p i c", c=NN),
                            ps[:].rearrange("p (i c) -> p i c", c=NN),
                            cj[:, None, :].to_broadcast((128, SUB, NN)),
                            mybir.AluOpType.add,
                        )
                    Rcur = Rnew

                # Squarings: T <- T @ T; final result in fp32
                for k in range(S_SQ):
                    last = k == S_SQ - 1
                    if last:
                        Tnew = iop.tile([128, CW], F32, tag="out")
                    else:
                        Tnew = workp.tile([128, CW], F16, tag="R")
                    for sb in range(NSUB):
                        ps = psump.tile([128, SW], F32, tag="ps")
                        for i in range(SUB):
                            col = (sb * SUB + i) * NN
                            for g in range(4):
                                gp = slice(g * 32, (g + 1) * 32)
                                nc.tensor.matmul(
                                    ps[gp, i * NN:(i + 1) * NN],
                                    Rcur[gp, col:col + NN],
                                    Rcur[gp, col:col + NN],
                                    start=True, stop=True,
                                    tile_position=(g * 32, g * 32),
                                )
                        dst = Tnew[:, sb * SW:(sb + 1) * SW]
                        if sb % 2 == 0:
                            nc.scalar.copy(dst, ps[:])
                        else:
                            nc.vector.tensor_copy(dst, ps[:])
                    Rcur = Tnew

                nc.sync.dma_start(y_dram[:, ch * CW:(ch + 1) * CW], Rcur[:])

    nc.compile()
    return nc


def _host_consts():
    c = np.zeros((128, (M_TAYLOR + 1) * NN), np.float16)
    eye = np.eye(NN, dtype=np.float16)
    for j in range(M_TAYLOR + 1):
        aj = np.float16(1.0 / math.factorial(j))
        for g in range(4):
            c[g * 32:(g + 1) * 32, j * NN:(j + 1) * NN] = aj * eye
    return c


_NC_CACHE = {}
_LAST_RESULTS = None


def _get_nc(nchunks):
    if nchunks not in _NC_CACHE:
        _NC_CACHE[nchunks] = build_nc(nchunks)
    return _NC_CACHE[nchunks]


def kernel(P):
    P = np.ascontiguousarray(np.asarray(P), dtype=np.float32)
    assert P.shape == (B, H, NN, NN)
    nchunks = NGRP // CHUNK_G

    # per-core layout: matrix m = g*NGRP + t -> partitions 32g+r, col 32t+c
    X = P.reshape(NCORES, 4, NGRP, NN, NN)
    Xdma = np.ascontiguousarray(
        X.transpose(0, 1, 3, 2, 4).reshape(NCORES, 128, NGRP * NN))

    consts = _host_consts()
    nc = _get_nc(nchunks)
    in_maps = [{"x": Xdma[c], "consts": consts} for c in range(NCORES)]
    trace = bool(int(os.environ.get("EXPEIG_TRACE", "0")))
    res = run_bass_kernel_spmd(nc, in_maps, list(range(NCORES)), trace=trace)
    global _LAST_RESULTS
    _LAST_RESULTS = res
    Y = np.stack([r["y"] for r in res.results])          # [cores, 128, NGRP*32]
    out = (Y.reshape(NCORES, 4, NN, NGRP, NN)
            .transpose(0, 1, 3, 2, 4)
            .reshape(B, H, NN, NN))
    return np.ascontiguousarray(out, dtype=np.float32)


if __name__ == "__main__":
    rng = np.random.default_rng(0)
    A = rng.standard_normal((B, H, NN, NN)).astype(np.float32)
    P = np.einsum("bhij,bhkj->bhik", A, A) / NN + 1e-3 * np.eye(NN, np.float32)
    P = 0.5 * (P + np.swapaxes(P, -1, -2))
    out = kernel(P=P)
    print("kernel ran, out shape", out.shape)
